# revision 18
# baseline (speedup 1.0000x reference)
"""Blahut-Arimoto VQ quantizer on 8 trn2 NeuronCores (bf16-delta formulation).

Problem (hardcoded): z [32, 64, 32, 32] f32, codebook W [1024, 64], pi uniform
[1024], beta scalar. N = 32768 rows, K = 1024 codes, D = 64. Data-parallel
over N: core c owns rows [4096c, 4096c+4096).

Math: distances are never materialized. With G = z_flat @ W.T, the row factor
exp(-beta |z|^2) cancels in every BA softmax, so the kernel works with
  At[n, k] = exp(2 beta G[n, k] - beta e2[k])        (e2 = rowwise |W|^2)
and, since At = 1 + A' with |A'| <~ 0.03 here, stores only the DELTA
A' = At - 1 in bf16 (the delta keeps the full signal; At itself in bf16
would lose it under the leading 1). Per BA iteration (Q0 = pi):
  y[n]  = SQ + sum_k Qb[k] A'[k, n]                  (PE, bf16, 1 cyc/row)
  u     = 1/y,   u' = u - 1 (bf16)
  v[k]  = 4096 + Su' + c1[k] + sum_n A'[n, k] u'[n]  (PE over row-major A')
  AllReduce v (iters 1-4 on device; iter 5 exported, host finishes Q5)
  Q    <- clip(Q v / N, 1e-10)
(c1 = colsum A' precomputed once; SQ/Su' are exact f32 strip reductions.)

The argmax key is computed separately in FULL f32 on the PE:
  key[n, k] = s'[k] + sum_d z^T[d, n] (2 beta W^T)[d, k]
  s'[k] = ln(1024 Q4[k]) - beta e2[k]   (ln via series; 1024 Q4 ~ 1 +- 1e-4)
folded into one matmul via an augmented 65th contraction row (z row = 1,
rhs row 64 = s' written at runtime). Index = 1024 - sum((key>=rowmax)*(1024-k)).
z_q_soft^T = (1+u5') * (SW + A' @ Wq) is exported for the host-side losses;
z_q itself is the host gather W[idx] (value-identical to the straight-through
output within ~1e-10). The G matmuls feeding exp use f32r views (the PE's
reduced-precision fast mode) - their error sits below the bf16 quantization
of A', while the f32 key matmul keeps argmax noise ~100x below the
reference's own fp32 noise floor.
"""

import os
import sys

import numpy as np

_TRN_REPO = "/opt/trn_rl_repo"
if _TRN_REPO not in sys.path:
    sys.path.insert(0, _TRN_REPO)

import concourse.bass as bass  # noqa: E402
import concourse.bacc as bacc  # noqa: E402
import concourse.mybir as mybir  # noqa: E402
from concourse import tile  # noqa: E402

F32 = mybir.dt.float32
F32R = mybir.dt.float32r
BF16 = mybir.dt.bfloat16
FP16 = mybir.dt.float16
Alu = mybir.AluOpType
Act = mybir.ActivationFunctionType
AxT = mybir.AxisListType

N_CORES = 8
N_TOTAL = 32768
N_LOC = N_TOTAL // N_CORES  # 4096
K = 1024
D = 64
KT = K // 128  # 8 k-tiles
NT = N_LOC // 128  # 32 n-tiles
NCHUNK = 512
NCH = N_LOC // NCHUNK  # 8
EPS = 1e-10
BA_ITERS = 5


def _emit(nc, tc):
    # ---- kernel I/O ----
    zaug = nc.dram_tensor("zaug", [D + 1, N_LOC], F32, kind="ExternalInput")
    rhsaug = nc.dram_tensor("rhsaug", [D + 1, K], F32, kind="ExternalInput")
    wrows = nc.dram_tensor("wrows", [K, D], F32, kind="ExternalInput")
    pi_col = nc.dram_tensor("pi_col", [128, KT], F32, kind="ExternalInput")
    pi_colb = nc.dram_tensor("pi_colb", [128, KT], BF16, kind="ExternalInput")
    pi_strip = nc.dram_tensor("pi_strip", [1, K], F32, kind="ExternalInput")
    iotan = nc.dram_tensor("iotan", [128, K], FP16, kind="ExternalInput")

    zqst_out = nc.dram_tensor("zqst", [D, N_LOC], F32, kind="ExternalOutput")
    idxs_out = nc.dram_tensor("idxs", [128, NT], F32, kind="ExternalOutput")
    q4_out = nc.dram_tensor("q4", [128, KT], F32, kind="ExternalOutput")
    v5_out = nc.dram_tensor("v5", [1, K], F32, kind="ExternalOutput")

    groups = [list(range(N_CORES))]

    sb_cm = tc.tile_pool(name="sb", bufs=1)
    sb = sb_cm.__enter__()
    sb2_cm = tc.tile_pool(name="sb2", bufs=2)
    sb2 = sb2_cm.__enter__()
    dram_cm = tc.tile_pool(name="dram", bufs=1, space="DRAM")
    dram = dram_cm.__enter__()

    # ---- resident SBUF ----
    L2 = [sb.tile([128, N_LOC], BF16, tag=f"L2_{t}", name=f"L2_{t}") for t in range(KT)]
    L1 = [sb.tile([128, K], BF16, tag=f"L1_{t}", name=f"L1_{t}") for t in range(NT)]
    zaug_sb = sb.tile([D + 1, N_LOC], F32, tag="zaug")
    rhsaug_sb = sb.tile([D + 1, K], F32, tag="rhsaug")
    uprow = sb.tile([1, N_LOC], BF16, tag="uprow")
    ucol = sb.tile([128, NT], BF16, tag="ucol")
    iota_b = sb.tile([128, K], FP16, tag="iota")
    wqb = sb.tile([128, KT * D], BF16, tag="wqb")
    con = sb.tile([128, 128], F32, tag="con")
    pi_c = con[:, 0:8]
    qa = con[:, 8:16]
    qb = con[:, 16:24]
    vcol = con[:, 24:32]
    mcol = con[:, 32:33]
    acol = con[:, 33:34]
    idx_sb = con[:, 34:66]
    swcol = con[0:D, 66:67]
    conb = sb.tile([128, 64], BF16, tag="conb")
    pib_c = conb[:, 0:8]
    qab = conb[:, 8:16]
    qbb = conb[:, 16:24]
    onesb_col = conb[:, 24:25]
    idb = conb[0:1, 25:26]  # bf16 1.0 for PE transpose identity
    strips = sb.tile([1, 4352], F32, tag="strips")
    q_strip = strips[0:1, 0:K]
    vs_strip = strips[0:1, K : 2 * K]
    ser = strips[0:1, 2 * K : 3 * K]
    c1_strip = strips[0:1, 3 * K : 4 * K]
    sq_cell = strips[0:1, 4 * K : 4 * K + 1]
    su_cell = strips[0:1, 4 * K + 1 : 4 * K + 2]
    sw_row = strips[0:1, 4 * K + 8 : 4 * K + 8 + D]
    onesb_row = sb.tile([1, NCHUNK], BF16, tag="onesrow")

    v_in = [dram.tile([1, K], F32, tag=f"vin{t}", name=f"vin{t}") for t in range(4)]
    v_ar = [dram.tile([1, K], F32, tag=f"var{t}", name=f"var{t}") for t in range(4)]

    # ---- constant loads ----
    nc.sync.dma_start(zaug_sb[:, :], zaug[:, :])
    nc.sync.dma_start(rhsaug_sb[:, :], rhsaug[:, :])
    nc.sync.dma_start(pi_c, pi_col[:, :])
    nc.sync.dma_start(pib_c, pi_colb[:, :])
    nc.sync.dma_start(q_strip, pi_strip[:, :])
    nc.sync.dma_start(iota_b[:, :], iotan[:, :])
    nc.vector.memset(onesb_col, 1.0)
    nc.vector.memset(idb, 1.0)
    nc.vector.memset(onesb_row[:, :], 1.0)
    nc.vector.memset(strips[0:1, 4 * K + 2 : 4 * K + 3], 1.0)

    ps0_cm = tc.tile_pool(name="ps0", bufs=1, space="PSUM")
    ps0 = ps0_cm.__enter__()


    # ---- L2' = exp(G^T + bias) - 1 (bf16): contract over D+1 (row 64 adds
    # -beta*e2 via the ones row of zaug) ----
    for t in range(KT):
        for c in range(NCH):
            g = ps0.tile([128, NCHUNK], F32, tag="big", bufs=4)
            nc.tensor.matmul(
                g[:, :],
                rhsaug_sb[:, t * 128 : (t + 1) * 128],
                zaug_sb[:, c * NCHUNK : (c + 1) * NCHUNK],
                start=True,
                stop=True,
            )
            nc.scalar.activation(g[:, :], g[:, :], Act.Exp)
            nc.vector.tensor_scalar_add(
                L2[t][:, c * NCHUNK : (c + 1) * NCHUNK], g[:, :], -1.0
            )

    # ---- L1' = row-major copy, same construction ----
    for n in range(NT):
        g = ps0.tile([128, K], F32, tag="big", bufs=4)
        for h in range(2):
            nc.tensor.matmul(
                g[:, h * NCHUNK : (h + 1) * NCHUNK],
                zaug_sb[:, n * 128 : (n + 1) * 128],
                rhsaug_sb[:, h * NCHUNK : (h + 1) * NCHUNK],
                start=True,
                stop=True,
            )
        nc.scalar.activation(g[:, :], g[:, :], Act.Exp)
        nc.vector.tensor_scalar_add(L1[n][:, :], g[:, :], -1.0)

    ps0_cm.__exit__(None, None, None)
    ps_cm = tc.tile_pool(name="ps", bufs=1, space="PSUM")
    ps = ps_cm.__enter__()

    # ---- c1[k] = colsum of A' ----
    c1p = ps.tile([1, K], F32, tag="vrow")
    for n in range(NT):
        for h in range(2):
            nc.tensor.matmul(
                c1p[:, h * NCHUNK : (h + 1) * NCHUNK],
                onesb_col,
                L1[n][:, h * NCHUNK : (h + 1) * NCHUNK],
                start=(n == 0),
                stop=(n == NT - 1),
            )
    nc.vector.tensor_copy(c1_strip, c1p[:, :])

    # ---- BA iterations ----
    qf = [pi_c, qa, qb, qa, qb]
    qbf = [pib_c, qab, qbb, qab, qbb]
    for it in range(BA_ITERS):
        nc.vector.tensor_reduce(sq_cell, q_strip, AxT.X, Alu.add)
        # y' per 512-chunk; u = 1/(SQ + y'); u' = u - 1 -> uprow bf16
        for c in range(NCH):
            y = ps.tile([1, NCHUNK], F32, tag="y", bufs=4)
            for t in range(KT):
                nc.tensor.matmul(
                    y[:, :],
                    qbf[it][:, t : t + 1],
                    L2[t][:, c * NCHUNK : (c + 1) * NCHUNK],
                    start=(t == 0),
                    stop=(t == KT - 1),
                )
            us = sb2.tile([1, NCHUNK], F32, tag="ustage")
            nc.vector.tensor_scalar(us[:, :], y[:, :], sq_cell, None, Alu.add)
            nc.vector.reciprocal(us[:, :], us[:, :])
            nc.vector.tensor_scalar_add(
                uprow[0:1, c * NCHUNK : (c + 1) * NCHUNK], us[:, :], -1.0
            )
        # u' columns via PE transpose of [1, 128] slices
        ucp = ps.tile([128, 2 * NT], BF16, tag="uc", bufs=2)
        for n in range(NT):
            nc.tensor.transpose(
                ucp[:, 2 * n : 2 * n + 1], uprow[0:1, n * 128 : (n + 1) * 128], idb
            )
        nc.vector.tensor_copy(ucol[:, :], ucp[:, 0 : 2 * NT : 2])
        # v'[k] = sum_n A'[n, k] u'[n]
        vp = ps.tile([1, K], F32, tag="vrow")
        for n in range(NT):
            for h in range(2):
                nc.tensor.matmul(
                    vp[:, h * NCHUNK : (h + 1) * NCHUNK],
                    ucol[:, n : n + 1],
                    L1[n][:, h * NCHUNK : (h + 1) * NCHUNK],
                    start=(n == 0),
                    stop=(n == NT - 1),
                )
        # v = 4096 + Su' + c1 + v'   (Su' = ones^T @ ucol, one tiny MM)
        sup = ps.tile([1, NT], F32, tag="y", bufs=4)
        nc.tensor.matmul(sup[:, :], onesb_col, ucol[:, :], start=True, stop=True)
        nc.vector.tensor_reduce(su_cell, sup[:, :], AxT.X, Alu.add)
        nc.vector.tensor_scalar_add(su_cell, su_cell, float(N_LOC))
        nc.vector.tensor_tensor(vs_strip, c1_strip, vp[:, :], Alu.add)
        nc.vector.tensor_scalar(vs_strip, vs_strip, su_cell, None, Alu.add)

        if it < 4:
            nc.gpsimd.dma_start(v_in[it][:, :], vs_strip)
            nc.gpsimd.collective_compute(
                "AllReduce",
                Alu.add,
                replica_groups=groups,
                ins=[v_in[it][:, :].opt()],
                outs=[v_ar[it][:, :].opt()],
            )
            nc.gpsimd.dma_start(vs_strip, v_ar[it][:, :])
            nc.vector.scalar_tensor_tensor(
                q_strip, vs_strip, 1.0 / N_TOTAL, q_strip, Alu.mult, Alu.mult
            )
            nc.vector.tensor_scalar_max(q_strip, q_strip, EPS)
            # column layout for the y-matmul lhsT: vcol[p, t] = v[128 t + p]
            nc.gpsimd.dma_start(
                vcol,
                v_ar[it][:, :]
                .rearrange("o (t p) -> o t p", p=128)
                .rearrange("o t p -> (o p) t"),
            )
            q_new = qf[it + 1]
            nc.vector.scalar_tensor_tensor(
                q_new, vcol, 1.0 / N_TOTAL, qf[it], Alu.mult, Alu.mult
            )
            nc.vector.tensor_scalar_max(q_new, q_new, EPS)
            nc.vector.tensor_copy(qbf[it + 1], q_new)
        else:
            nc.sync.dma_start(v5_out[:, :], vs_strip)

    q4 = qf[4]
    nc.sync.dma_start(q4_out[:, :], q4)

    # ---- s' = ln(1024 Q4) - beta e2 -> rhsaug row 64 (replacing -beta e2) ----
    w = vs_strip  # dead
    nc.vector.tensor_scalar(w, q_strip, 1024.0, -1.0, Alu.mult, Alu.add)
    nc.vector.tensor_scalar(ser, w, -0.25, 1.0 / 3.0, Alu.mult, Alu.add)
    nc.vector.tensor_tensor(ser, ser, w, Alu.mult)
    nc.vector.tensor_scalar_add(ser, ser, -0.5)
    nc.vector.tensor_tensor(ser, ser, w, Alu.mult)
    nc.vector.tensor_scalar_add(ser, ser, 1.0)
    nc.vector.tensor_tensor(ser, ser, w, Alu.mult)
    # add -beta*e2 (read back from the dram input; c1_strip is dead) and
    # DMA the finished s' into rhsaug row 64 (cross-partition move)
    nc.sync.dma_start(c1_strip, rhsaug[D : D + 1, :])
    nc.vector.tensor_tensor(ser, ser, c1_strip, Alu.add)
    nc.sync.dma_start(rhsaug_sb[D : D + 1, :], ser)

    # ---- Wq = Q4 * W rows (bf16); SW = colsum Wq ----
    for t in range(KT):
        wst = sb2.tile([128, D], F32, tag="wst")
        nc.sync.dma_start(wst[:, :], wrows[t * 128 : (t + 1) * 128, :])
        nc.vector.tensor_scalar(
            wqb[:, t * D : (t + 1) * D], wst[:, :], q4[:, t : t + 1], None, Alu.mult
        )
    swp = ps.tile([1, D], F32, tag="y", bufs=4)
    for t in range(KT):
        nc.tensor.matmul(
            swp[:, :],
            onesb_col,
            wqb[:, t * D : (t + 1) * D],
            start=(t == 0),
            stop=(t == KT - 1),
        )
    nc.vector.tensor_copy(sw_row, swp[:, :])
    swtp = ps.tile([D, 1], F32, tag="uc", bufs=2)
    nc.tensor.transpose(swtp[:, :], sw_row, strips[0:1, 4 * K + 2 : 4 * K + 3])
    nc.vector.tensor_copy(swcol, swtp[:, :])

    ps_cm.__exit__(None, None, None)
    psf_cm = tc.tile_pool(name="psf", bufs=2, space="PSUM")
    psf = psf_cm.__enter__()

    # ---- z_q_soft^T = (1 + u5') * (SW + A' @ Wq) ----
    for c in range(NCH):
        zq_ps = psf.tile([D, NCHUNK], F32, tag="zq")
        for t in range(KT):
            nc.tensor.matmul(
                zq_ps[:, :],
                wqb[:, t * D : (t + 1) * D],
                L2[t][:, c * NCHUNK : (c + 1) * NCHUNK],
                start=(t == 0),
                stop=(t == KT - 1),
            )
        ubf = psf.tile([128, NCHUNK], F32, tag="ubf")
        nc.tensor.matmul(
            ubf[:, :],
            onesb_row[0:1, 0:128],
            uprow[0:1, c * NCHUNK : (c + 1) * NCHUNK],
            start=True,
            stop=False,
        )
        nc.tensor.matmul(
            ubf[:, :], onesb_row[0:1, 0:128], onesb_row[:, :], start=False, stop=True
        )
        zq_sb = sb2.tile([D, NCHUNK], F32, tag="zqsb")
        nc.vector.tensor_scalar(zq_sb[:, :], zq_ps[:, :], swcol, None, Alu.add)
        nc.vector.tensor_tensor(zq_sb[:, :], zq_sb[:, :], ubf[0:D, :], Alu.mult)
        nc.sync.dma_start(zqst_out[:, c * NCHUNK : (c + 1) * NCHUNK], zq_sb[:, :])

    # ---- argmax keys (full f32; s' folded in via the augmented row) ----
    kz = zaug_sb
    kr = rhsaug_sb
    for n in range(NT):
        key = psf.tile([128, K], F32, tag="key")
        for h in range(2):
            nc.tensor.matmul(
                key[:, h * NCHUNK : (h + 1) * NCHUNK],
                kz[:, n * 128 : (n + 1) * 128],
                kr[:, h * NCHUNK : (h + 1) * NCHUNK],
                start=True,
                stop=True,
            )
        nc.vector.tensor_reduce(mcol, key[:, :], AxT.X, Alu.max)
        nc.vector.scalar_tensor_tensor(
            key[:, :],
            key[:, :],
            mcol,
            iota_b[:, :],
            Alu.is_ge,
            Alu.mult,
            accum_out=acol,
        )
        nc.vector.tensor_scalar(
            idx_sb[:, n : n + 1], acol, -1.0, 1024.0, Alu.mult, Alu.add
        )
    nc.sync.dma_start(idxs_out[:, :], idx_sb)

    psf_cm.__exit__(None, None, None)
    dram_cm.__exit__(None, None, None)
    sb2_cm.__exit__(None, None, None)
    sb_cm.__exit__(None, None, None)


_CACHE = {}


def _build():
    if "nc" in _CACHE:
        return _CACHE["nc"]
    nc = bacc.Bacc("TRN2", target_bir_lowering=False, debug=False, num_devices=N_CORES)
    with tile.TileContext(nc) as tc:
        _emit(nc, tc)
    nc.compile()
    _CACHE["nc"] = nc
    return nc


def _host_inputs(z, emb_weight, pi, beta):
    f32 = np.float32
    bf16 = mybir.dt.np(BF16)
    z = np.asarray(z, f32)
    W = np.asarray(emb_weight, f32)
    pi = np.asarray(pi, f32)
    b = float(np.asarray(beta))

    wt2b = (2.0 * b * W.T).astype(f32)  # [64, 1024]
    e2 = np.sum(W.astype(f32) ** 2, axis=1, dtype=f32)
    nbe2 = (-b * e2).astype(f32)
    rhsaug = np.ascontiguousarray(np.concatenate([wt2b, nbe2[None, :]], axis=0))
    iotan = np.ascontiguousarray(
        np.tile((1024.0 - np.arange(K, dtype=f32))[None, :], (128, 1)).astype(
            np.float16
        )
    )

    def col(v, dt=f32):
        return np.ascontiguousarray(v.reshape(KT, 128).T.astype(dt))

    common = {
        "rhsaug": rhsaug,
        "wrows": np.ascontiguousarray(W),
        "pi_col": col(pi),
        "pi_colb": col(pi, bf16),
        "pi_strip": np.ascontiguousarray(pi[None, :]),
        "iotan": iotan,
    }
    in_maps = []
    for c in range(N_CORES):
        zc = z[4 * c : 4 * c + 4]
        zt = zc.transpose(1, 0, 2, 3).reshape(D, N_LOC)
        zaug = np.ascontiguousarray(
            np.concatenate([zt, np.ones((1, N_LOC), f32)], axis=0)
        )
        m = dict(common)
        m["zaug"] = zaug
        in_maps.append(m)
    return in_maps


def _postprocess(results, z, emb_weight, beta):
    f32 = np.float32
    W = np.asarray(emb_weight, f32)
    z = np.asarray(z, f32)

    idx_parts, zqs_parts = [], []
    v5 = np.zeros(K, f32)
    for c in range(N_CORES):
        r = results[c]
        idx_parts.append(np.asarray(r["idxs"], f32).T.reshape(-1))
        zqs_parts.append(np.asarray(r["zqst"], f32).T)
        v5 = v5 + np.asarray(r["v5"], f32).reshape(-1)
    q4 = np.asarray(results[0]["q4"], f32).T.reshape(-1)

    idx = np.clip(np.rint(np.concatenate(idx_parts)), 0, K - 1).astype(np.int32)
    zqs = np.concatenate(zqs_parts, axis=0)  # [32768, 64]

    zq = W[idx].reshape(32, 32, 32, D).transpose(0, 3, 1, 2).astype(f32)

    zf = z.transpose(0, 2, 3, 1).reshape(-1, D)
    X = f32(np.mean((zf.astype(np.float64) - zqs.astype(np.float64)) ** 2))
    commitment = f32(f32(0.25) * X)
    codebook = X

    q5 = np.maximum(v5 * f32(1.0 / N_TOTAL) * q4, f32(EPS)).astype(f32)
    entropy = -np.sum(q5 * np.log(q5), dtype=f32)
    entropy_loss = f32(f32(-0.01) * entropy)

    return (zq, commitment, codebook, entropy_loss, idx)


def kernel(z, emb_weight, pi, beta):
    nc = _build()
    in_maps = _host_inputs(z, emb_weight, pi, beta)
    if os.environ.get("KERNEL_SIM"):
        from concourse import bass_interp

        sim = bass_interp.MultiCoreSim(nc, N_CORES)
        for c in range(N_CORES):
            for name, arr in in_maps[c].items():
                sim.cores[c].tensor(name)[:] = arr
        sim.simulate(check_with_hw=False)
        results = [
            {
                k: np.array(sim.cores[c].mem_tensor(k))
                for k in ("idxs", "zqst", "q4", "v5")
            }
            for c in range(N_CORES)
        ]
    else:
        from concourse.bass_utils import run_bass_kernel_spmd

        kr = run_bass_kernel_spmd(
            nc,
            in_maps,
            core_ids=list(range(N_CORES)),
            trace=bool(os.environ.get("KERNEL_TRACE")),
        )
        results = kr.results
        kernel.last_exec_time_ns = kr.exec_time_ns
    return _postprocess(results, z, emb_weight, beta)


kernel.last_exec_time_ns = None


# revision 19
# speedup vs baseline: 1.0436x; 1.0436x over previous
"""Blahut-Arimoto VQ quantizer on 8 trn2 NeuronCores (bf16-delta formulation).

Problem (hardcoded): z [32, 64, 32, 32] f32, codebook W [1024, 64], pi uniform
[1024], beta scalar. N = 32768 rows, K = 1024 codes, D = 64. Data-parallel
over N: core c owns rows [4096c, 4096c+4096).

Math: distances are never materialized. With G = z_flat @ W.T, the row factor
exp(-beta |z|^2) cancels in every BA softmax, so the kernel works with
  At[n, k] = exp(2 beta G[n, k] - beta e2[k])        (e2 = rowwise |W|^2)
and, since At = 1 + A' with |A'| <~ 0.03 here, stores only the DELTA
A' = At - 1 in bf16 (the delta keeps the full signal; At itself in bf16
would lose it under the leading 1). Per BA iteration (Q0 = pi):
  y[n]  = SQ + sum_k Qb[k] A'[k, n]                  (PE, bf16, 1 cyc/row)
  u     = 1/y,   u' = u - 1 (bf16)
  v[k]  = 4096 + Su' + c1[k] + sum_n A'[n, k] u'[n]  (PE over row-major A')
  AllReduce v (iters 1-4 on device; iter 5 exported, host finishes Q5)
  Q    <- clip(Q v / N, 1e-10)
(c1 = colsum A' precomputed once; SQ/Su' are exact f32 strip reductions.)

The argmax key is computed separately in FULL f32 on the PE:
  key[n, k] = s'[k] + sum_d z^T[d, n] (2 beta W^T)[d, k]
  s'[k] = ln(1024 Q4[k]) - beta e2[k]   (ln via series; 1024 Q4 ~ 1 +- 1e-4)
folded into one matmul via an augmented 65th contraction row (z row = 1,
rhs row 64 = s' written at runtime). Index = 1024 - sum((key>=rowmax)*(1024-k)).
z_q_soft^T = (1+u5') * (SW + A' @ Wq) is exported for the host-side losses;
z_q itself is the host gather W[idx] (value-identical to the straight-through
output within ~1e-10). The G matmuls feeding exp use f32r views (the PE's
reduced-precision fast mode) - their error sits below the bf16 quantization
of A', while the f32 key matmul keeps argmax noise ~100x below the
reference's own fp32 noise floor.
"""

import os
import sys

import numpy as np

_TRN_REPO = "/opt/trn_rl_repo"
if _TRN_REPO not in sys.path:
    sys.path.insert(0, _TRN_REPO)

import concourse.bass as bass  # noqa: E402
import concourse.bacc as bacc  # noqa: E402
import concourse.mybir as mybir  # noqa: E402
from concourse import tile  # noqa: E402

F32 = mybir.dt.float32
F32R = mybir.dt.float32r
BF16 = mybir.dt.bfloat16
FP16 = mybir.dt.float16
Alu = mybir.AluOpType
Act = mybir.ActivationFunctionType
AxT = mybir.AxisListType

N_CORES = 8
N_TOTAL = 32768
N_LOC = N_TOTAL // N_CORES  # 4096
K = 1024
D = 64
KT = K // 128  # 8 k-tiles
NT = N_LOC // 128  # 32 n-tiles
NCHUNK = 512
NCH = N_LOC // NCHUNK  # 8
EPS = 1e-10
BA_ITERS = 5


def _emit(nc, tc):
    # ---- kernel I/O ----
    zaug = nc.dram_tensor("zaug", [D + 1, N_LOC], F32, kind="ExternalInput")
    rhsaug = nc.dram_tensor("rhsaug", [D + 1, K], F32, kind="ExternalInput")
    wrows = nc.dram_tensor("wrows", [K, D], F32, kind="ExternalInput")
    pi_col = nc.dram_tensor("pi_col", [128, KT], F32, kind="ExternalInput")
    pi_colb = nc.dram_tensor("pi_colb", [128, KT], BF16, kind="ExternalInput")
    pi_strip = nc.dram_tensor("pi_strip", [1, K], F32, kind="ExternalInput")
    iotan = nc.dram_tensor("iotan", [128, K], FP16, kind="ExternalInput")
    zaugb = nc.dram_tensor("zaugb", [D + 1, N_LOC], BF16, kind="ExternalInput")
    rhsaugb = nc.dram_tensor("rhsaugb", [D + 1, K], BF16, kind="ExternalInput")

    zqst_out = nc.dram_tensor("zqst", [D, N_LOC], F32, kind="ExternalOutput")
    idxs_out = nc.dram_tensor("idxs", [128, NT], F32, kind="ExternalOutput")
    q4_out = nc.dram_tensor("q4", [128, KT], F32, kind="ExternalOutput")
    v5_out = nc.dram_tensor("v5", [1, K], F32, kind="ExternalOutput")

    groups = [list(range(N_CORES))]

    sb_cm = tc.tile_pool(name="sb", bufs=1)
    sb = sb_cm.__enter__()
    sb2_cm = tc.tile_pool(name="sb2", bufs=2)
    sb2 = sb2_cm.__enter__()
    dram_cm = tc.tile_pool(name="dram", bufs=1, space="DRAM")
    dram = dram_cm.__enter__()

    # ---- resident SBUF ----
    L2 = [sb.tile([128, N_LOC], BF16, tag=f"L2_{t}", name=f"L2_{t}") for t in range(KT)]
    L1 = [sb.tile([128, K], BF16, tag=f"L1_{t}", name=f"L1_{t}") for t in range(NT)]
    zaug_sb = sb.tile([D + 1, N_LOC], F32, tag="zaug")
    rhsaug_sb = sb.tile([D + 1, K], F32, tag="rhsaug")
    zaugb_sb = sb.tile([D + 1, N_LOC], BF16, tag="zaugb")
    rhsaugb_sb = sb.tile([D + 1, K], BF16, tag="rhsaugb")
    uprow = sb.tile([1, N_LOC], BF16, tag="uprow")
    ucol = sb.tile([128, NT], BF16, tag="ucol")
    iota_b = sb.tile([128, K], FP16, tag="iota")
    wqb = sb.tile([128, KT * D], BF16, tag="wqb")
    con = sb.tile([128, 128], F32, tag="con")
    pi_c = con[:, 0:8]
    qa = con[:, 8:16]
    qb = con[:, 16:24]
    vcol = con[:, 24:32]
    mcol = con[:, 32:33]
    acol = con[:, 33:34]
    idx_sb = con[:, 34:66]
    swcol = con[0:D, 66:67]
    conb = sb.tile([128, 64], BF16, tag="conb")
    pib_c = conb[:, 0:8]
    qab = conb[:, 8:16]
    qbb = conb[:, 16:24]
    onesb_col = conb[:, 24:25]
    idb = conb[0:1, 25:26]  # bf16 1.0 for PE transpose identity
    strips = sb.tile([1, 4224], F32, tag="strips")
    q_strip = strips[0:1, 0:K]
    vs_strip = strips[0:1, K : 2 * K]
    ser = strips[0:1, 2 * K : 3 * K]
    c1_strip = strips[0:1, 3 * K : 4 * K]
    sq_cell = strips[0:1, 4 * K : 4 * K + 1]
    su_cell = strips[0:1, 4 * K + 1 : 4 * K + 2]
    sw_row = strips[0:1, 4 * K + 8 : 4 * K + 8 + D]
    onesb_row = sb.tile([1, NCHUNK], BF16, tag="onesrow")

    v_in = [dram.tile([1, K], F32, tag=f"vin{t}", name=f"vin{t}") for t in range(4)]
    v_ar = [dram.tile([1, K], F32, tag=f"var{t}", name=f"var{t}") for t in range(4)]

    # ---- constant loads ----
    nc.sync.dma_start(zaug_sb[:, :], zaug[:, :])
    nc.sync.dma_start(rhsaug_sb[:, :], rhsaug[:, :])
    nc.sync.dma_start(zaugb_sb[:, :], zaugb[:, :])
    nc.sync.dma_start(rhsaugb_sb[:, :], rhsaugb[:, :])
    nc.sync.dma_start(pi_c, pi_col[:, :])
    nc.sync.dma_start(pib_c, pi_colb[:, :])
    nc.sync.dma_start(q_strip, pi_strip[:, :])
    nc.sync.dma_start(iota_b[:, :], iotan[:, :])
    nc.vector.memset(onesb_col, 1.0)
    nc.vector.memset(idb, 1.0)
    nc.vector.memset(onesb_row[:, :], 1.0)
    nc.vector.memset(strips[0:1, 4 * K + 2 : 4 * K + 3], 1.0)

    ps0_cm = tc.tile_pool(name="ps0", bufs=1, space="PSUM")
    ps0 = ps0_cm.__enter__()


    # ---- L2' = exp(G^T + bias) - 1 (bf16): contract over D+1 (row 64 adds
    # -beta*e2 via the ones row of zaug) ----
    for t in range(KT):
        for c in range(NCH):
            g = ps0.tile([128, NCHUNK], F32, tag="big", bufs=4)
            nc.tensor.matmul(
                g[:, :],
                rhsaugb_sb[:, t * 128 : (t + 1) * 128],
                zaugb_sb[:, c * NCHUNK : (c + 1) * NCHUNK],
                start=True,
                stop=True,
            )
            nc.scalar.activation(g[:, :], g[:, :], Act.Exp)
            nc.vector.tensor_scalar_add(
                L2[t][:, c * NCHUNK : (c + 1) * NCHUNK], g[:, :], -1.0
            )

    # ---- L1' = row-major copy, same construction ----
    for n in range(NT):
        g = ps0.tile([128, K], F32, tag="big", bufs=4)
        for h in range(2):
            nc.tensor.matmul(
                g[:, h * NCHUNK : (h + 1) * NCHUNK],
                zaugb_sb[:, n * 128 : (n + 1) * 128],
                rhsaugb_sb[:, h * NCHUNK : (h + 1) * NCHUNK],
                start=True,
                stop=True,
            )
        nc.scalar.activation(g[:, :], g[:, :], Act.Exp)
        nc.vector.tensor_scalar_add(L1[n][:, :], g[:, :], -1.0)

    ps0_cm.__exit__(None, None, None)
    ps_cm = tc.tile_pool(name="ps", bufs=1, space="PSUM")
    ps = ps_cm.__enter__()

    # ---- c1[k] = colsum of A' ----
    c1p = ps.tile([1, K], F32, tag="vrow")
    for n in range(NT):
        for h in range(2):
            nc.tensor.matmul(
                c1p[:, h * NCHUNK : (h + 1) * NCHUNK],
                onesb_col,
                L1[n][:, h * NCHUNK : (h + 1) * NCHUNK],
                start=(n == 0),
                stop=(n == NT - 1),
            )
    nc.vector.tensor_copy(c1_strip, c1p[:, :])

    # ---- BA iterations ----
    qf = [pi_c, qa, qb, qa, qb]
    qbf = [pib_c, qab, qbb, qab, qbb]
    for it in range(BA_ITERS):
        nc.vector.tensor_reduce(sq_cell, q_strip, AxT.X, Alu.add)
        # y' per 512-chunk; u = 1/(SQ + y'); u' = u - 1 -> uprow bf16
        for c in range(NCH):
            y = ps.tile([1, NCHUNK], F32, tag="y", bufs=4)
            for t in range(KT):
                nc.tensor.matmul(
                    y[:, :],
                    qbf[it][:, t : t + 1],
                    L2[t][:, c * NCHUNK : (c + 1) * NCHUNK],
                    start=(t == 0),
                    stop=(t == KT - 1),
                )
            us = sb2.tile([1, NCHUNK], F32, tag="ustage", bufs=1)
            nc.vector.tensor_scalar(us[:, :], y[:, :], sq_cell, None, Alu.add)
            nc.vector.reciprocal(us[:, :], us[:, :])
            nc.vector.tensor_scalar_add(
                uprow[0:1, c * NCHUNK : (c + 1) * NCHUNK], us[:, :], -1.0
            )
        # u' columns via PE transpose of [1, 128] slices
        ucp = ps.tile([128, 2 * NT], BF16, tag="uc", bufs=2)
        for n in range(NT):
            nc.tensor.transpose(
                ucp[:, 2 * n : 2 * n + 1], uprow[0:1, n * 128 : (n + 1) * 128], idb
            )
        nc.vector.tensor_copy(ucol[:, :], ucp[:, 0 : 2 * NT : 2])
        # v'[k] = sum_n A'[n, k] u'[n]
        vp = ps.tile([1, K], F32, tag="vrow")
        for n in range(NT):
            for h in range(2):
                nc.tensor.matmul(
                    vp[:, h * NCHUNK : (h + 1) * NCHUNK],
                    ucol[:, n : n + 1],
                    L1[n][:, h * NCHUNK : (h + 1) * NCHUNK],
                    start=(n == 0),
                    stop=(n == NT - 1),
                )
        # v = 4096 + Su' + c1 + v'   (Su' = ones^T @ ucol, one tiny MM)
        sup = ps.tile([1, NT], F32, tag="y", bufs=4)
        nc.tensor.matmul(sup[:, :], onesb_col, ucol[:, :], start=True, stop=True)
        nc.vector.tensor_reduce(su_cell, sup[:, :], AxT.X, Alu.add)
        nc.vector.tensor_scalar_add(su_cell, su_cell, float(N_LOC))
        nc.vector.tensor_tensor(vs_strip, c1_strip, vp[:, :], Alu.add)
        nc.vector.tensor_scalar(vs_strip, vs_strip, su_cell, None, Alu.add)

        if it < 4:
            nc.gpsimd.dma_start(v_in[it][:, :], vs_strip)
            nc.gpsimd.collective_compute(
                "AllReduce",
                Alu.add,
                replica_groups=groups,
                ins=[v_in[it][:, :].opt()],
                outs=[v_ar[it][:, :].opt()],
            )
            nc.gpsimd.dma_start(vs_strip, v_ar[it][:, :])
            nc.vector.scalar_tensor_tensor(
                q_strip, vs_strip, 1.0 / N_TOTAL, q_strip, Alu.mult, Alu.mult
            )
            nc.vector.tensor_scalar_max(q_strip, q_strip, EPS)
            # column layout for the y-matmul lhsT: vcol[p, t] = v[128 t + p]
            nc.gpsimd.dma_start(
                vcol,
                v_ar[it][:, :]
                .rearrange("o (t p) -> o t p", p=128)
                .rearrange("o t p -> (o p) t"),
            )
            q_new = qf[it + 1]
            nc.vector.scalar_tensor_tensor(
                q_new, vcol, 1.0 / N_TOTAL, qf[it], Alu.mult, Alu.mult
            )
            nc.vector.tensor_scalar_max(q_new, q_new, EPS)
            nc.vector.tensor_copy(qbf[it + 1], q_new)
        else:
            nc.sync.dma_start(v5_out[:, :], vs_strip)

    q4 = qf[4]
    nc.sync.dma_start(q4_out[:, :], q4)

    # ---- s' = ln(1024 Q4) - beta e2 -> rhsaug row 64 (replacing -beta e2) ----
    w = vs_strip  # dead
    nc.vector.tensor_scalar(w, q_strip, 1024.0, -1.0, Alu.mult, Alu.add)
    nc.vector.tensor_scalar(ser, w, -0.25, 1.0 / 3.0, Alu.mult, Alu.add)
    nc.vector.tensor_tensor(ser, ser, w, Alu.mult)
    nc.vector.tensor_scalar_add(ser, ser, -0.5)
    nc.vector.tensor_tensor(ser, ser, w, Alu.mult)
    nc.vector.tensor_scalar_add(ser, ser, 1.0)
    nc.vector.tensor_tensor(ser, ser, w, Alu.mult)
    # add -beta*e2 (read back from the dram input; c1_strip is dead) and
    # DMA the finished s' into rhsaug row 64 (cross-partition move)
    nc.sync.dma_start(c1_strip, rhsaug[D : D + 1, :])
    nc.vector.tensor_tensor(ser, ser, c1_strip, Alu.add)
    nc.sync.dma_start(rhsaug_sb[D : D + 1, :], ser)

    # ---- Wq = Q4 * W rows (bf16); SW = colsum Wq ----
    for t in range(KT):
        wst = sb2.tile([128, D], F32, tag="wst", bufs=1)
        nc.sync.dma_start(wst[:, :], wrows[t * 128 : (t + 1) * 128, :])
        nc.vector.tensor_scalar(
            wqb[:, t * D : (t + 1) * D], wst[:, :], q4[:, t : t + 1], None, Alu.mult
        )
    swp = ps.tile([1, D], F32, tag="y", bufs=4)
    for t in range(KT):
        nc.tensor.matmul(
            swp[:, :],
            onesb_col,
            wqb[:, t * D : (t + 1) * D],
            start=(t == 0),
            stop=(t == KT - 1),
        )
    nc.vector.tensor_copy(sw_row, swp[:, :])
    swtp = ps.tile([D, 1], F32, tag="uc", bufs=2)
    nc.tensor.transpose(swtp[:, :], sw_row, strips[0:1, 4 * K + 2 : 4 * K + 3])
    nc.vector.tensor_copy(swcol, swtp[:, :])

    ps_cm.__exit__(None, None, None)
    psf_cm = tc.tile_pool(name="psf", bufs=2, space="PSUM")
    psf = psf_cm.__enter__()

    # ---- z_q_soft^T = (1 + u5') * (SW + A' @ Wq) ----
    for c in range(NCH):
        zq_ps = psf.tile([D, NCHUNK], F32, tag="zq")
        for t in range(KT):
            nc.tensor.matmul(
                zq_ps[:, :],
                wqb[:, t * D : (t + 1) * D],
                L2[t][:, c * NCHUNK : (c + 1) * NCHUNK],
                start=(t == 0),
                stop=(t == KT - 1),
            )
        ubf = psf.tile([128, NCHUNK], F32, tag="ubf")
        nc.tensor.matmul(
            ubf[:, :],
            onesb_row[0:1, 0:128],
            uprow[0:1, c * NCHUNK : (c + 1) * NCHUNK],
            start=True,
            stop=False,
        )
        nc.tensor.matmul(
            ubf[:, :], onesb_row[0:1, 0:128], onesb_row[:, :], start=False, stop=True
        )
        zq_sb = sb2.tile([D, NCHUNK], BF16, tag="zqsb")
        nc.vector.tensor_scalar(zq_sb[:, :], zq_ps[:, :], swcol, None, Alu.add)
        nc.vector.tensor_tensor(zq_sb[:, :], zq_sb[:, :], ubf[0:D, :], Alu.mult)
        nc.gpsimd.dma_start(zqst_out[:, c * NCHUNK : (c + 1) * NCHUNK], zq_sb[:, :])

    # ---- argmax keys (full f32; s' folded in via the augmented row) ----
    kz = zaug_sb
    kr = rhsaug_sb
    for n in range(NT):
        key = psf.tile([128, K], F32, tag="key")
        for h in range(2):
            nc.tensor.matmul(
                key[:, h * NCHUNK : (h + 1) * NCHUNK],
                kz[:, n * 128 : (n + 1) * 128],
                kr[:, h * NCHUNK : (h + 1) * NCHUNK],
                start=True,
                stop=True,
            )
        nc.vector.tensor_reduce(mcol, key[:, :], AxT.X, Alu.max)
        nc.vector.scalar_tensor_tensor(
            key[:, :],
            key[:, :],
            mcol,
            iota_b[:, :],
            Alu.is_ge,
            Alu.mult,
            accum_out=acol,
        )
        nc.vector.tensor_scalar(
            idx_sb[:, n : n + 1], acol, -1.0, 1024.0, Alu.mult, Alu.add
        )
    nc.sync.dma_start(idxs_out[:, :], idx_sb)

    psf_cm.__exit__(None, None, None)
    dram_cm.__exit__(None, None, None)
    sb2_cm.__exit__(None, None, None)
    sb_cm.__exit__(None, None, None)


_CACHE = {}


def _build():
    if "nc" in _CACHE:
        return _CACHE["nc"]
    nc = bacc.Bacc("TRN2", target_bir_lowering=False, debug=False, num_devices=N_CORES)
    with tile.TileContext(nc) as tc:
        _emit(nc, tc)
    nc.compile()
    _CACHE["nc"] = nc
    return nc


def _host_inputs(z, emb_weight, pi, beta):
    f32 = np.float32
    bf16 = mybir.dt.np(BF16)
    z = np.asarray(z, f32)
    W = np.asarray(emb_weight, f32)
    pi = np.asarray(pi, f32)
    b = float(np.asarray(beta))

    wt2b = (2.0 * b * W.T).astype(f32)  # [64, 1024]
    e2 = np.sum(W.astype(f32) ** 2, axis=1, dtype=f32)
    nbe2 = (-b * e2).astype(f32)
    rhsaug = np.ascontiguousarray(np.concatenate([wt2b, nbe2[None, :]], axis=0))
    iotan = np.ascontiguousarray(
        np.tile((1024.0 - np.arange(K, dtype=f32))[None, :], (128, 1)).astype(
            np.float16
        )
    )

    def col(v, dt=f32):
        return np.ascontiguousarray(v.reshape(KT, 128).T.astype(dt))

    common = {
        "rhsaug": rhsaug,
        "rhsaugb": rhsaug.astype(bf16),
        "wrows": np.ascontiguousarray(W),
        "pi_col": col(pi),
        "pi_colb": col(pi, bf16),
        "pi_strip": np.ascontiguousarray(pi[None, :]),
        "iotan": iotan,
    }
    in_maps = []
    for c in range(N_CORES):
        zc = z[4 * c : 4 * c + 4]
        zt = zc.transpose(1, 0, 2, 3).reshape(D, N_LOC)
        zaug = np.ascontiguousarray(
            np.concatenate([zt, np.ones((1, N_LOC), f32)], axis=0)
        )
        m = dict(common)
        m["zaug"] = zaug
        m["zaugb"] = zaug.astype(bf16)
        in_maps.append(m)
    return in_maps


def _postprocess(results, z, emb_weight, beta):
    f32 = np.float32
    W = np.asarray(emb_weight, f32)
    z = np.asarray(z, f32)

    idx_parts, zqs_parts = [], []
    v5 = np.zeros(K, f32)
    for c in range(N_CORES):
        r = results[c]
        idx_parts.append(np.asarray(r["idxs"], f32).T.reshape(-1))
        zqs_parts.append(np.asarray(r["zqst"], f32).T)
        v5 = v5 + np.asarray(r["v5"], f32).reshape(-1)
    q4 = np.asarray(results[0]["q4"], f32).T.reshape(-1)

    idx = np.clip(np.rint(np.concatenate(idx_parts)), 0, K - 1).astype(np.int32)
    zqs = np.concatenate(zqs_parts, axis=0)  # [32768, 64]

    zq = W[idx].reshape(32, 32, 32, D).transpose(0, 3, 1, 2).astype(f32)

    zf = z.transpose(0, 2, 3, 1).reshape(-1, D)
    X = f32(np.mean((zf.astype(np.float64) - zqs.astype(np.float64)) ** 2))
    commitment = f32(f32(0.25) * X)
    codebook = X

    q5 = np.maximum(v5 * f32(1.0 / N_TOTAL) * q4, f32(EPS)).astype(f32)
    entropy = -np.sum(q5 * np.log(q5), dtype=f32)
    entropy_loss = f32(f32(-0.01) * entropy)

    return (zq, commitment, codebook, entropy_loss, idx)


def kernel(z, emb_weight, pi, beta):
    nc = _build()
    in_maps = _host_inputs(z, emb_weight, pi, beta)
    if os.environ.get("KERNEL_SIM"):
        from concourse import bass_interp

        sim = bass_interp.MultiCoreSim(nc, N_CORES)
        for c in range(N_CORES):
            for name, arr in in_maps[c].items():
                sim.cores[c].tensor(name)[:] = arr
        sim.simulate(check_with_hw=False)
        results = [
            {
                k: np.array(sim.cores[c].mem_tensor(k))
                for k in ("idxs", "zqst", "q4", "v5")
            }
            for c in range(N_CORES)
        ]
    else:
        from concourse.bass_utils import run_bass_kernel_spmd

        kr = run_bass_kernel_spmd(
            nc,
            in_maps,
            core_ids=list(range(N_CORES)),
            trace=bool(os.environ.get("KERNEL_TRACE")),
        )
        results = kr.results
        kernel.last_exec_time_ns = kr.exec_time_ns
    return _postprocess(results, z, emb_weight, beta)


kernel.last_exec_time_ns = None
